# revision 13
# baseline (speedup 1.0000x reference)
"""GRU cell on 8 Trainium2 NeuronCores.

Reference computation (B=65536, D=256):
    z = sigmoid(x@Wz + h@Uz + bz)
    r = sigmoid(x@Wr + h@Ur + br)
    h_hat = tanh(x@Wh + (r*h)@Uh + bh)
    h_t = z*h + (1-z)*h_hat  ; returns (h_t, h_t)

Strategy: data-parallel over the batch dim (8 shards of 8192 rows).

Per-core kernel (PE-roofline oriented; the warm Tensor engine streams
one 512-col bf16 matmul every ~216 ns, so 384 matmuls = ~83 us is the
floor and everything else must hide behind it):
- Host pre-packs every tensor into [128, free] bf16 layout:
  x/h shards as [128, 16 chunks x (2 hidden-halves x 512 batch)] so
  each chunk is one contiguous 256 KB DMA; weights as one
  [128, 12x256] pack (gate-major, then output-half-major, so the first
  128 KB DMA unblocks the first matmul group); biases [128, 6] f32.
- All six GEMMs run as bf16 matmuls accumulating f32 in PSUM.
  PSUM budget: P_r (2 banks) + P_z (2) + P_h double-buffered (4) = 8.
- Software pipeline: iteration i issues PE groups r(i), h(i-1), z(i).
  The candidate-gate matmuls consume rh = r*h from the *previous*
  iteration, so the PE stream never waits on the current chunk's
  ACT/DVE results -> no PE gaps -> HAM stays at K=8/8 (2.4 GHz).
- Startup: input DMAs are split across both HWDGE queues (sync +
  scalar), ordered by first-use time, and two full-width warm-up
  matmuls on the first weight tile keep the PE HAM activity window
  busy while the first x/h chunk lands.
- ACT applies bias+sigmoid/tanh straight out of PSUM (same table set
  -> one table load); DVE does the 4 elementwise ops per chunk in
  bf16 2x mode.  The last chunk's combine is split per output half to
  shorten the drain tail.
"""

import os
import sys

for _p in ("/opt/trn_rl_repo", "/root/.axon_site/_ro/trn_rl_repo"):
    if os.path.isdir(_p) and _p not in sys.path:
        sys.path.append(_p)

import numpy as np
import ml_dtypes

BF16 = ml_dtypes.bfloat16

B = 65536
D = 256
N_CORES = 8
S = B // N_CORES  # 8192 batch rows per core
CB = 512  # batch columns per chunk
NCH = S // CB  # 16 chunks
WARMUP_MMS = 2  # full-width PE warm-up matmuls before the real stream


def build_nc():
    import concourse.mybir as mybir
    import concourse.tile as tile
    from concourse import bacc

    f32 = mybir.dt.float32
    bf16 = mybir.dt.bfloat16
    AF = mybir.ActivationFunctionType

    nc = bacc.Bacc("TRN2", target_bir_lowering=False)
    # x and h interleaved per chunk so one 512 KB DMA fetches both:
    # chunk c occupies cols [c*2048, (c+1)*2048): x in [0:1024) (k-half
    # major, batch minor), h in [1024:2048)
    xhH = nc.dram_tensor("xhH", [128, NCH * 4 * CB], bf16, kind="ExternalInput")
    # weight pack col = gate*1024 + g*512 + w_i*128,
    # gate in (r, z, h); g = output half; w_i in (W k0, W k1, U k0, U k1)
    wAll = nc.dram_tensor("wAll", [128, 12 * 256], bf16, kind="ExternalInput")
    # bias pack cols: [br g0, br g1, bz g0, bz g1, bh g0, bh g1]
    bAll = nc.dram_tensor("bAll", [128, 6], f32, kind="ExternalInput")
    oH = nc.dram_tensor("oH", [128, 2, S], bf16, kind="ExternalOutput")

    with tile.TileContext(nc) as tc:
        with (
            tc.tile_pool(name="const", bufs=1) as cpool,
            tc.tile_pool(name="inp", bufs=1) as ipool,
            tc.tile_pool(name="work", bufs=1) as wpool,
            tc.tile_pool(name="psum", bufs=1, space="PSUM") as ppool,
        ):
            # --- startup DMAs, split across both HWDGE queues and ordered
            # by first-use time.  sync: x/h chunks 0 (as two halves so the
            # r-gate W matmuls can start before the h half lands) and 1.
            # scalar: biases, weights, then a dummy activation to preload
            # the sigmoid/tanh table set during the DMA window.
            bt = cpool.tile([128, 6], f32, tag="bias")
            nc.scalar.dma_start(bt[:], bAll[:])

            w_sb = {}
            for gate in ("r", "z", "h"):
                w_sb[gate] = cpool.tile(
                    [128, 4 * 256], bf16, tag=f"w_{gate}", name=f"w_{gate}"
                )

            xts, hts, rhs_t, zts = {}, {}, {}, {}
            xh_tiles = {}

            def load_chunk(c, eng, split=False, gate_from=None):
                xh = ipool.tile([128, 4 * CB], bf16, tag="xh", bufs=4)
                # Startup pacing: a 1-elem copy from the previous chunk's
                # h-half makes this DMA wait until that chunk has fully
                # landed, so prefetches don't steal SDMA bandwidth from
                # the critical first loads (queued DMAs round-robin at
                # packet granularity, so un-gated prefetch delays all).
                if gate_from is not None:
                    nc.vector.tensor_copy(
                        xh[0:1, 0:1], gate_from[0:1, 2 * CB : 2 * CB + 1]
                    )
                base = c * 4 * CB
                if split:
                    eng.dma_start(xh[:, 0:1024], xhH[:, base : base + 1024])
                    eng.dma_start(xh[:, 1024:2048], xhH[:, base + 1024 : base + 2048])
                else:
                    eng.dma_start(xh[:], xhH[:, base : base + 4 * CB])
                xh_tiles[c] = xh
                xts[c], hts[c] = xh[:, 0 : 2 * CB], xh[:, 2 * CB : 4 * CB]

            # walrus hoists the two ~1.3 us ACT_TABLE_LOADs to the front
            # of the Scalar queue, so every scalar-issued DMA is delayed
            # by ~2.6 us: critical-path DMAs (r weights, chunk 0) go on
            # sync, interleaved in first-use order; bt/wz/wh ride scalar.
            nc.sync.dma_start(w_sb["r"][:, 0:512], wAll[:, 0:512])
            xh0 = ipool.tile([128, 4 * CB], bf16, tag="xh", bufs=4, name="xh0")
            nc.sync.dma_start(xh0[:, 0:1024], xhH[:, 0:1024])
            nc.sync.dma_start(xh0[:, 1024:2048], xhH[:, 1024:2048])
            nc.sync.dma_start(w_sb["r"][:, 512:1024], wAll[:, 512:1024])
            xh_tiles[0] = xh0
            xts[0], hts[0] = xh0[:, 0 : 2 * CB], xh0[:, 2 * CB : 4 * CB]
            load_chunk(1, nc.sync, gate_from=xh_tiles[0])
            load_chunk(2, nc.sync, gate_from=xh_tiles[1])
            # z/h weights aren't needed until ~mid-iteration 0 / iteration
            # 1 — gate them off the critical startup window too.
            nc.vector.tensor_copy(
                w_sb["z"][0:1, 0:1], xh_tiles[0][0:1, 2 * CB : 2 * CB + 1]
            )
            nc.scalar.dma_start(w_sb["z"][:], wAll[:, 1024:2048])
            nc.vector.tensor_copy(
                w_sb["h"][0:1, 0:1], xh_tiles[1][0:1, 2 * CB : 2 * CB + 1]
            )
            nc.scalar.dma_start(w_sb["h"][:], wAll[:, 2048:3072])

            # --- PE warm-up: full-width matmuls on the r weight tile
            # (same stationary operand; results discarded).  Keeps the
            # HAM activity window busy while x/h chunk 0 lands.
            pwarm = ppool.tile([128, 2 * CB], f32, tag="p_r")
            for _ in range(WARMUP_MMS):
                nc.tensor.matmul(
                    pwarm[:, 0:CB],
                    w_sb["r"][:, 0:128],
                    w_sb["r"][:, 0:512],
                    start=True,
                    stop=True,
                )

            def mm_group(p, g, wt, rhs_w, rhs_u):
                """p[:, g*CB:(g+1)*CB] = W[:,g].T@rhs_w + U[:,g].T@rhs_u."""
                out = p[:, g * CB : (g + 1) * CB]
                for j, (w_i, rhs) in enumerate(
                    ((0, rhs_w), (1, rhs_w), (2, rhs_u), (3, rhs_u))
                ):
                    lhsT = wt[:, g * 512 + w_i * 128 : g * 512 + (w_i + 1) * 128]
                    k = w_i % 2
                    nc.tensor.matmul(
                        out,
                        lhsT,
                        rhs[:, k * CB : (k + 1) * CB],
                        start=(j == 0),
                        stop=(j == 3),
                    )

            for i in range(NCH + 1):
                if 1 <= i and i + 2 < NCH:
                    load_chunk(i + 2, nc.sync)

                # --- PE stream: r(i), h(i-1), z(i) ---
                if i < NCH:
                    xt, ht = xts[i], hts[i]
                    p_r = ppool.tile([128, 2 * CB], f32, tag="p_r")
                    mm_group(p_r, 0, w_sb["r"], xt, ht)
                    mm_group(p_r, 1, w_sb["r"], xt, ht)
                if i >= 1:
                    c = i - 1
                    p_h = ppool.tile([128, 2 * CB], f32, tag="p_h", bufs=2)
                    mm_group(p_h, 0, w_sb["h"], xts[c], rhs_t[c])
                    mm_group(p_h, 1, w_sb["h"], xts[c], rhs_t[c])
                if i < NCH:
                    p_z = ppool.tile([128, 2 * CB], f32, tag="p_z")
                    mm_group(p_z, 0, w_sb["z"], xt, ht)
                    mm_group(p_z, 1, w_sb["z"], xt, ht)

                # --- ACT: sigmoid(r) -> DVE: rh (feeds next iter's PE) ---
                if i < NCH:
                    rt = wpool.tile([128, 2 * CB], bf16, tag="rt", bufs=2)
                    nc.scalar.activation(
                        rt[:, 0:CB], p_r[:, 0:CB], AF.Sigmoid, bias=bt[:, 0:1]
                    )
                    nc.scalar.activation(
                        rt[:, CB:], p_r[:, CB:], AF.Sigmoid, bias=bt[:, 1:2]
                    )
                    rh = wpool.tile([128, 2 * CB], bf16, tag="rh", bufs=2)
                    nc.vector.tensor_mul(rh[:], rt[:], ht[:])
                    rhs_t[i] = rh

                # --- ACT: tanh -> DVE combine -> store for chunk i-1 ---
                if i >= 1:
                    c = i - 1
                    hh = wpool.tile([128, 2 * CB], bf16, tag="hh", bufs=2)
                    t1 = wpool.tile([128, 2 * CB], bf16, tag="t1", bufs=2)
                    t2 = wpool.tile([128, 2 * CB], bf16, tag="t2", bufs=2)
                    o = wpool.tile([128, 2 * CB], bf16, tag="o", bufs=2)
                    ht_c = hts[c]
                    # last chunk: per-half pipeline to shorten the tail
                    parts = (0, 1) if c == NCH - 1 else (None,)
                    for part in parts:
                        if part is None:
                            sl = slice(0, 2 * CB)
                            nc.scalar.activation(
                                hh[:, 0:CB], p_h[:, 0:CB], AF.Tanh, bias=bt[:, 4:5]
                            )
                            nc.scalar.activation(
                                hh[:, CB:], p_h[:, CB:], AF.Tanh, bias=bt[:, 5:6]
                            )
                        else:
                            sl = slice(part * CB, (part + 1) * CB)
                            nc.scalar.activation(
                                hh[:, sl], p_h[:, sl], AF.Tanh,
                                bias=bt[:, 4 + part : 5 + part],
                            )
                        nc.vector.tensor_sub(t1[:, sl], ht_c[:, sl], hh[:, sl])
                        nc.vector.tensor_mul(t2[:, sl], zts[c][:, sl], t1[:, sl])
                        nc.vector.tensor_add(o[:, sl], hh[:, sl], t2[:, sl])
                        if part is None:
                            nc.sync.dma_start(
                                oH[:, :, c * CB : (c + 1) * CB], o[:]
                            )
                        else:
                            nc.sync.dma_start(
                                oH[:, part, c * CB : (c + 1) * CB], o[:, sl]
                            )

                # --- ACT: sigmoid(z) (consumed next iteration by DVE) ---
                if i < NCH:
                    zt = wpool.tile([128, 2 * CB], bf16, tag="zt", bufs=3)
                    nc.scalar.activation(
                        zt[:, 0:CB], p_z[:, 0:CB], AF.Sigmoid, bias=bt[:, 2:3]
                    )
                    nc.scalar.activation(
                        zt[:, CB:], p_z[:, CB:], AF.Sigmoid, bias=bt[:, 3:4]
                    )
                    zts[i] = zt

    nc.compile()
    return nc


_NC_CACHE = {}


def _get_nc():
    if "nc" not in _NC_CACHE:
        _NC_CACHE["nc"] = build_nc()
    return _NC_CACHE["nc"]


def _pack_inputs(inputs):
    f32 = np.float32
    x = np.asarray(inputs["x"], f32)
    h = np.asarray(inputs["h_t_1"], f32)

    # weight pack [128, 12*256]: col = gate*1024 + g*512 + w_i*128
    blocks = []
    for wn, un in (("Wr", "Ur"), ("Wz", "Uz"), ("Wh", "Uh")):
        W = np.asarray(inputs[wn], f32)
        U = np.asarray(inputs[un], f32)
        for g in range(2):
            gs = slice(g * 128, (g + 1) * 128)
            blocks += [W[0:128, gs], W[128:256, gs], U[0:128, gs], U[128:256, gs]]
    wAll = np.ascontiguousarray(np.concatenate(blocks, axis=1).astype(BF16))

    bAll = np.empty((128, 6), f32)
    for gi, bn in enumerate(("br", "bz", "bh")):
        b = np.asarray(inputs[bn], f32)
        bAll[:, 2 * gi] = b[0:128]
        bAll[:, 2 * gi + 1] = b[128:256]

    consts = {"wAll": wAll, "bAll": np.ascontiguousarray(bAll)}

    def pack_xh(xs, hs):
        # each [S, 256] -> [128, NCH, 1024] with col = k*512 + j, then
        # interleave x/h per chunk -> [128, NCH*2048]
        def p(a):
            return a.astype(BF16).reshape(NCH, CB, 2, 128).transpose(3, 0, 2, 1)

        xh = np.concatenate([p(xs), p(hs)], axis=2)  # [128, NCH, 2048]
        return np.ascontiguousarray(xh.reshape(128, NCH * 4 * CB))

    in_maps = []
    for c in range(N_CORES):
        sl = slice(c * S, (c + 1) * S)
        m = {"xhH": pack_xh(x[sl], h[sl])}
        m.update(consts)
        in_maps.append(m)
    return in_maps


def run(inputs, trace=False):
    """Run on hardware; returns (h_t ndarray, BassKernelResults)."""
    from concourse.bass_utils import run_bass_kernel_spmd

    nc = _get_nc()
    in_maps = _pack_inputs(inputs)
    res = run_bass_kernel_spmd(nc, in_maps, list(range(N_CORES)), trace=trace)
    out = np.empty((B, D), np.float32)
    for c in range(N_CORES):
        oH = res.results[c]["oH"]  # [128, 2, S] bf16
        out[c * S : (c + 1) * S] = (
            oH.transpose(2, 1, 0).reshape(S, D).astype(np.float32)
        )
    return out, res


def kernel(**inputs):
    out, _ = run(inputs, trace=False)
    return (out, out)


# revision 14
# speedup vs baseline: 1.0295x; 1.0295x over previous
"""GRU cell on 8 Trainium2 NeuronCores.

Reference computation (B=65536, D=256):
    z = sigmoid(x@Wz + h@Uz + bz)
    r = sigmoid(x@Wr + h@Ur + br)
    h_hat = tanh(x@Wh + (r*h)@Uh + bh)
    h_t = z*h + (1-z)*h_hat  ; returns (h_t, h_t)

Strategy: data-parallel over the batch dim (8 shards of 8192 rows).

Per-core kernel (PE-roofline oriented; the warm Tensor engine streams
one 512-col bf16 matmul every ~216 ns, so 384 matmuls = ~83 us is the
floor and everything else must hide behind it):
- Host pre-packs every tensor into [128, free] bf16 layout:
  x/h shards as [128, 16 chunks x (2 hidden-halves x 512 batch)] so
  each chunk is one contiguous 256 KB DMA; weights as one
  [128, 12x256] pack (gate-major, then output-half-major, so the first
  128 KB DMA unblocks the first matmul group); biases [128, 6] f32.
- All six GEMMs run as bf16 matmuls accumulating f32 in PSUM.
  PSUM budget: P_r (2 banks) + P_z (2) + P_h double-buffered (4) = 8.
- Software pipeline: iteration i issues PE groups r(i), h(i-1), z(i).
  The candidate-gate matmuls consume rh = r*h from the *previous*
  iteration, so the PE stream never waits on the current chunk's
  ACT/DVE results -> no PE gaps -> HAM stays at K=8/8 (2.4 GHz).
- Startup: input DMAs are split across both HWDGE queues (sync +
  scalar), ordered by first-use time, and two full-width warm-up
  matmuls on the first weight tile keep the PE HAM activity window
  busy while the first x/h chunk lands.
- ACT applies bias+sigmoid/tanh straight out of PSUM (same table set
  -> one table load); DVE does the 4 elementwise ops per chunk in
  bf16 2x mode.  The last chunk's combine is split per output half to
  shorten the drain tail.
"""

import os
import sys

for _p in ("/opt/trn_rl_repo", "/root/.axon_site/_ro/trn_rl_repo"):
    if os.path.isdir(_p) and _p not in sys.path:
        sys.path.append(_p)

import numpy as np
import ml_dtypes

BF16 = ml_dtypes.bfloat16

B = 65536
D = 256
N_CORES = 8
S = B // N_CORES  # 8192 batch rows per core
CB = 512  # batch columns per chunk
NCH = S // CB  # 16 chunks
WARMUP_MMS = 2  # full-width PE warm-up matmuls before the real stream


def build_nc():
    import concourse.mybir as mybir
    import concourse.tile as tile
    from concourse import bacc

    f32 = mybir.dt.float32
    bf16 = mybir.dt.bfloat16
    AF = mybir.ActivationFunctionType

    nc = bacc.Bacc("TRN2", target_bir_lowering=False)
    # x and h interleaved per chunk so one 512 KB DMA fetches both:
    # chunk c occupies cols [c*2048, (c+1)*2048): x in [0:1024) (k-half
    # major, batch minor), h in [1024:2048)
    xhH = nc.dram_tensor("xhH", [128, NCH * 4 * CB], bf16, kind="ExternalInput")
    # weight pack col = gate*1024 + g*512 + w_i*128,
    # gate in (r, z, h); g = output half; w_i in (W k0, W k1, U k0, U k1)
    wAll = nc.dram_tensor("wAll", [128, 12 * 256], bf16, kind="ExternalInput")
    # bias pack cols: [br g0, br g1, bz g0, bz g1, bh g0, bh g1]
    bAll = nc.dram_tensor("bAll", [128, 6], f32, kind="ExternalInput")
    oH = nc.dram_tensor("oH", [128, 2, S], bf16, kind="ExternalOutput")

    with tile.TileContext(nc) as tc:
        with (
            tc.tile_pool(name="const", bufs=1) as cpool,
            tc.tile_pool(name="inp", bufs=1) as ipool,
            tc.tile_pool(name="work", bufs=1) as wpool,
            tc.tile_pool(name="psum", bufs=1, space="PSUM") as ppool,
        ):
            # --- startup DMAs, split across both HWDGE queues and ordered
            # by first-use time.  sync: x/h chunks 0 (as two halves so the
            # r-gate W matmuls can start before the h half lands) and 1.
            # scalar: biases, weights, then a dummy activation to preload
            # the sigmoid/tanh table set during the DMA window.
            bt = cpool.tile([128, 6], f32, tag="bias")
            nc.scalar.dma_start(bt[:], bAll[:])

            w_sb = {}
            for gate in ("r", "z", "h"):
                w_sb[gate] = cpool.tile(
                    [128, 4 * 256], bf16, tag=f"w_{gate}", name=f"w_{gate}"
                )

            xts, hts, rhs_t, zts = {}, {}, {}, {}
            xh_tiles = {}

            def load_chunk(c, eng):
                xh = ipool.tile([128, 4 * CB], bf16, tag="xh", bufs=4)
                eng.dma_start(xh[:], xhH[:, c * 4 * CB : (c + 1) * 4 * CB])
                xh_tiles[c] = xh
                xts[c], hts[c] = xh[:, 0 : 2 * CB], xh[:, 2 * CB : 4 * CB]

            # All startup DMAs ride the sync queue in exact first-use
            # order: a single queue row drains FIFO across the 16 SDMA
            # engine slots, so completions arrive in issue order at full
            # bandwidth with no round-robin contention.  (The scalar
            # queue is unusable early: walrus hoists the two ~1.3 us
            # ACT_TABLE_LOADs to its front.)  bt (3 KB) stays on scalar.
            nc.sync.dma_start(w_sb["r"][:, 0:512], wAll[:, 0:512])
            xh0 = ipool.tile([128, 4 * CB], bf16, tag="xh", bufs=4, name="xh0")
            nc.sync.dma_start(xh0[:, 0:1024], xhH[:, 0:1024])
            nc.sync.dma_start(xh0[:, 1024:2048], xhH[:, 1024:2048])
            nc.sync.dma_start(w_sb["r"][:, 512:1024], wAll[:, 512:1024])
            xh_tiles[0] = xh0
            xts[0], hts[0] = xh0[:, 0 : 2 * CB], xh0[:, 2 * CB : 4 * CB]
            nc.sync.dma_start(w_sb["z"][:], wAll[:, 1024:2048])
            load_chunk(1, nc.sync)
            nc.sync.dma_start(w_sb["h"][:], wAll[:, 2048:3072])
            load_chunk(2, nc.sync)

            # --- PE warm-up: full-width matmuls on the r weight tile
            # (same stationary operand; results discarded).  Keeps the
            # HAM activity window busy while x/h chunk 0 lands.
            pwarm = ppool.tile([128, 2 * CB], f32, tag="p_r")
            for _ in range(WARMUP_MMS):
                nc.tensor.matmul(
                    pwarm[:, 0:CB],
                    w_sb["r"][:, 0:128],
                    w_sb["r"][:, 0:512],
                    start=True,
                    stop=True,
                )

            def mm_group(p, g, wt, rhs_w, rhs_u):
                """p[:, g*CB:(g+1)*CB] = W[:,g].T@rhs_w + U[:,g].T@rhs_u."""
                out = p[:, g * CB : (g + 1) * CB]
                for j, (w_i, rhs) in enumerate(
                    ((0, rhs_w), (1, rhs_w), (2, rhs_u), (3, rhs_u))
                ):
                    lhsT = wt[:, g * 512 + w_i * 128 : g * 512 + (w_i + 1) * 128]
                    k = w_i % 2
                    nc.tensor.matmul(
                        out,
                        lhsT,
                        rhs[:, k * CB : (k + 1) * CB],
                        start=(j == 0),
                        stop=(j == 3),
                    )

            for i in range(NCH + 1):
                if 1 <= i and i + 2 < NCH:
                    load_chunk(i + 2, nc.sync)

                # --- PE stream: r(i), h(i-1), z(i) ---
                if i < NCH:
                    xt, ht = xts[i], hts[i]
                    p_r = ppool.tile([128, 2 * CB], f32, tag="p_r")
                    mm_group(p_r, 0, w_sb["r"], xt, ht)
                    mm_group(p_r, 1, w_sb["r"], xt, ht)
                if i >= 1:
                    c = i - 1
                    p_h = ppool.tile([128, 2 * CB], f32, tag="p_h", bufs=2)
                    mm_group(p_h, 0, w_sb["h"], xts[c], rhs_t[c])
                    mm_group(p_h, 1, w_sb["h"], xts[c], rhs_t[c])
                if i < NCH:
                    p_z = ppool.tile([128, 2 * CB], f32, tag="p_z")
                    mm_group(p_z, 0, w_sb["z"], xt, ht)
                    mm_group(p_z, 1, w_sb["z"], xt, ht)

                # --- ACT: sigmoid(r) -> DVE: rh (feeds next iter's PE) ---
                if i < NCH:
                    rt = wpool.tile([128, 2 * CB], bf16, tag="rt", bufs=2)
                    nc.scalar.activation(
                        rt[:, 0:CB], p_r[:, 0:CB], AF.Sigmoid, bias=bt[:, 0:1]
                    )
                    nc.scalar.activation(
                        rt[:, CB:], p_r[:, CB:], AF.Sigmoid, bias=bt[:, 1:2]
                    )
                    rh = wpool.tile([128, 2 * CB], bf16, tag="rh", bufs=2)
                    nc.vector.tensor_mul(rh[:], rt[:], ht[:])
                    rhs_t[i] = rh

                # --- ACT: tanh -> DVE combine -> store for chunk i-1 ---
                if i >= 1:
                    c = i - 1
                    hh = wpool.tile([128, 2 * CB], bf16, tag="hh", bufs=2)
                    t1 = wpool.tile([128, 2 * CB], bf16, tag="t1", bufs=2)
                    t2 = wpool.tile([128, 2 * CB], bf16, tag="t2", bufs=2)
                    o = wpool.tile([128, 2 * CB], bf16, tag="o", bufs=2)
                    ht_c = hts[c]
                    # last chunk: per-half pipeline to shorten the tail
                    parts = (0, 1) if c == NCH - 1 else (None,)
                    for part in parts:
                        if part is None:
                            sl = slice(0, 2 * CB)
                            nc.scalar.activation(
                                hh[:, 0:CB], p_h[:, 0:CB], AF.Tanh, bias=bt[:, 4:5]
                            )
                            nc.scalar.activation(
                                hh[:, CB:], p_h[:, CB:], AF.Tanh, bias=bt[:, 5:6]
                            )
                        else:
                            sl = slice(part * CB, (part + 1) * CB)
                            nc.scalar.activation(
                                hh[:, sl], p_h[:, sl], AF.Tanh,
                                bias=bt[:, 4 + part : 5 + part],
                            )
                        nc.vector.tensor_sub(t1[:, sl], ht_c[:, sl], hh[:, sl])
                        nc.vector.tensor_mul(t2[:, sl], zts[c][:, sl], t1[:, sl])
                        nc.vector.tensor_add(o[:, sl], hh[:, sl], t2[:, sl])
                        if part is None:
                            nc.sync.dma_start(
                                oH[:, :, c * CB : (c + 1) * CB], o[:]
                            )
                        else:
                            nc.sync.dma_start(
                                oH[:, part, c * CB : (c + 1) * CB], o[:, sl]
                            )

                # --- ACT: sigmoid(z) (consumed next iteration by DVE) ---
                if i < NCH:
                    zt = wpool.tile([128, 2 * CB], bf16, tag="zt", bufs=3)
                    nc.scalar.activation(
                        zt[:, 0:CB], p_z[:, 0:CB], AF.Sigmoid, bias=bt[:, 2:3]
                    )
                    nc.scalar.activation(
                        zt[:, CB:], p_z[:, CB:], AF.Sigmoid, bias=bt[:, 3:4]
                    )
                    zts[i] = zt

    nc.compile()
    return nc


_NC_CACHE = {}


def _get_nc():
    if "nc" not in _NC_CACHE:
        _NC_CACHE["nc"] = build_nc()
    return _NC_CACHE["nc"]


def _pack_inputs(inputs):
    f32 = np.float32
    x = np.asarray(inputs["x"], f32)
    h = np.asarray(inputs["h_t_1"], f32)

    # weight pack [128, 12*256]: col = gate*1024 + g*512 + w_i*128
    blocks = []
    for wn, un in (("Wr", "Ur"), ("Wz", "Uz"), ("Wh", "Uh")):
        W = np.asarray(inputs[wn], f32)
        U = np.asarray(inputs[un], f32)
        for g in range(2):
            gs = slice(g * 128, (g + 1) * 128)
            blocks += [W[0:128, gs], W[128:256, gs], U[0:128, gs], U[128:256, gs]]
    wAll = np.ascontiguousarray(np.concatenate(blocks, axis=1).astype(BF16))

    bAll = np.empty((128, 6), f32)
    for gi, bn in enumerate(("br", "bz", "bh")):
        b = np.asarray(inputs[bn], f32)
        bAll[:, 2 * gi] = b[0:128]
        bAll[:, 2 * gi + 1] = b[128:256]

    consts = {"wAll": wAll, "bAll": np.ascontiguousarray(bAll)}

    def pack_xh(xs, hs):
        # each [S, 256] -> [128, NCH, 1024] with col = k*512 + j, then
        # interleave x/h per chunk -> [128, NCH*2048]
        def p(a):
            return a.astype(BF16).reshape(NCH, CB, 2, 128).transpose(3, 0, 2, 1)

        xh = np.concatenate([p(xs), p(hs)], axis=2)  # [128, NCH, 2048]
        return np.ascontiguousarray(xh.reshape(128, NCH * 4 * CB))

    in_maps = []
    for c in range(N_CORES):
        sl = slice(c * S, (c + 1) * S)
        m = {"xhH": pack_xh(x[sl], h[sl])}
        m.update(consts)
        in_maps.append(m)
    return in_maps


def run(inputs, trace=False):
    """Run on hardware; returns (h_t ndarray, BassKernelResults)."""
    from concourse.bass_utils import run_bass_kernel_spmd

    nc = _get_nc()
    in_maps = _pack_inputs(inputs)
    res = run_bass_kernel_spmd(nc, in_maps, list(range(N_CORES)), trace=trace)
    out = np.empty((B, D), np.float32)
    for c in range(N_CORES):
        oH = res.results[c]["oH"]  # [128, 2, S] bf16
        out[c * S : (c + 1) * S] = (
            oH.transpose(2, 1, 0).reshape(S, D).astype(np.float32)
        )
    return out, res


def kernel(**inputs):
    out, _ = run(inputs, trace=False)
    return (out, out)
